# revision 11
# baseline (speedup 1.0000x reference)
"""Trainium2 Bass kernel for nn_Concat_26147760898611.

Mean-pool over the word dim of article_concat [256, 2048, 300] and
options_concat [256, 64, 300], concat features -> [256, 600].

Sharding: pure data parallel over batch across 8 NeuronCores
(32 batches per core).

Per-core design (v4 — descriptor-deal engine balancing, fat descriptors):
  - Profiling shows SDMA engine 15 sustains ~22.4 GB/s vs ~26.1 GB/s for
    engines 0-14, so with a uniform layout engine 15 finishes ~30 us
    after the rest and owns the critical path.
  - HWDGE descriptor dealing (measured): a DMA with n descriptors is
    dealt to k = (largest divisor of n <= 16) engines, starting at
    engine 0, n/k consecutive descriptors each.  The SBUF partition a
    descriptor targets is irrelevant to which engine moves it.
  - So: all DMAs keep fat 19.2 KB per-partition descriptors, and
    SKIP_BATCHES article batches are loaded by a [120, 16, 300] DMA
    (120 = 15*8 -> engines 0-14 only) plus an [8, 16, 300] DMA
    (engines 0-7), which sheds exactly those batches' bytes from
    engine 15.  With 4 skip batches: engine 15 moves 28/33 units
    (199 us at 22.4 GB/s), engines 0-7 33 units + 4/8 extra descriptors
    (~197 us at 26.1 GB/s) — all engines finish together.
  - Each article batch [2048, 300] lands in an SBUF tile
    [128 partitions, 16 words, 300]; partition p holds 16 consecutive
    words (19.2 KB contiguous per partition).  The word axis folds
    16 -> 8 -> 4 on the VectorEngine (fp32-exact adds); 4 TensorEngine
    matmuls with a sliding one-hot selector reduce across partitions
    into PSUM row b.  Selector values are 1/2048 (1/64 for options), so
    PSUM holds the mean directly and the Scalar engine (and its ACT
    table preamble load) is never used; DVE copies PSUM -> out tile.
  - Options: partition p holds 16 consecutive words of batch p//4, one
    block-selector reduction, drained into the output tile early.
  - The last batch is split into shrinking chunks so the post-last-DMA
    tail (fold + matmul + copy + store) is short.
  - A burst of dummy matmuls at kernel start warms the PE HAM clock
    gate (1.2 -> 2.4 GHz) before real data lands.

Self-contained: hardcodes all shapes; no file reads.
"""

import numpy as np

N_CORES = 8
B = 256  # full batch
BC = B // N_CORES  # 32 batches per core
DIM = 300
AW = 2048  # article words per batch
OW = 64  # options words per batch
P = 128  # SBUF partitions
AWP = AW // P  # 16 article words per partition

SKIP_BATCHES = (6, 13, 20, 27)  # batches whose DMAs bypass engine 15
TAIL_CHUNKS = [8, 4, 2, 1, 1]  # geometric split of the final batch
DATA_BUFS = 6
FOLD_BUFS = 3
WARMUP_MMS = 12

_CACHE = {}


def _build_nc():
    import concourse.bacc as bacc
    import concourse.mybir as mybir
    import concourse.tile as tile

    f32 = mybir.dt.float32
    nc = bacc.Bacc("TRN2", target_bir_lowering=False, debug=False)

    art = nc.dram_tensor("article", [BC, AW, DIM], f32, kind="ExternalInput")
    opt = nc.dram_tensor("options", [BC, OW, DIM], f32, kind="ExternalInput")
    sel_a = nc.dram_tensor("sel_a", [P, 2 * BC - 1], f32, kind="ExternalInput")
    sel_o = nc.dram_tensor("sel_o", [P, BC], f32, kind="ExternalInput")
    out = nc.dram_tensor("out", [BC, 2 * DIM], f32, kind="ExternalOutput")

    # [BC, 128, 16, 300]: partition p <- words p*16 .. p*16+15 (contiguous)
    art_r = art.ap().rearrange("b (p w) f -> b p w f", p=P)
    # per-partition word view of the last batch: [128, 16, 300]
    art_last = art.ap()[BC - 1].rearrange("(p w) f -> p w f", p=P)
    # [128, 16, 300]: partition p <- 16 consecutive words of batch p//4
    opt_r = opt.ap().rearrange("b (s q) f -> (b s) q f", s=P // BC)

    with tile.TileContext(nc) as tc:
        with (
            tc.tile_pool(name="const", bufs=1) as cpool,
            tc.tile_pool(name="data", bufs=DATA_BUFS) as dpool,
            tc.tile_pool(name="fold", bufs=FOLD_BUFS) as fpool,
            tc.tile_pool(name="outp", bufs=1) as opool,
            tc.tile_pool(name="psum", bufs=1, space="PSUM") as ppool,
        ):
            sel_a_t = cpool.tile([P, 2 * BC - 1], f32, tag="sel_a")
            sel_o_t = cpool.tile([P, BC], f32, tag="sel_o")
            out_t = opool.tile([BC, 2 * DIM], f32, tag="out")

            psum_a = ppool.tile([BC, DIM], f32, tag="psum_a")
            psum_b = ppool.tile([BC, DIM], f32, tag="psum_b")
            psum_w = ppool.tile([BC, 2 * BC - 1], f32, tag="psum_w")

            def load_batch(b):
                """DMA article batch b; skip batches bypass engine 15."""
                t = dpool.tile([P, AWP, DIM], f32, tag="data")
                if b in SKIP_BATCHES:
                    # 120 descriptors -> engines 0-14; 8 -> engines 0-7.
                    # The [8] rides the scalar-engine HWDGE ring so the
                    # sync ring keeps its uniform 128-descriptor rhythm.
                    nc.sync.dma_start(t[0:120], art_r[b, 0:120])
                    nc.scalar.dma_start(t[120:P], art_r[b, 120:P])
                else:
                    nc.sync.dma_start(t[:], art_r[b])
                return t

            def reduce_tile(t, nch, sel_ap, psum, first, last):
                """Fold nch cols twice on DVE, then matmul-reduce into psum."""
                cur, n = t, nch
                for lvl in range(2):
                    if n == 1:
                        break
                    n //= 2
                    nxt = fpool.tile([P, n, DIM], f32, tag=f"fold{lvl}_{nch}")
                    nc.vector.tensor_add(nxt[:], cur[:, 0:n, :], cur[:, n : 2 * n, :])
                    cur = nxt
                for j in range(n):
                    nc.tensor.matmul(
                        psum[:], sel_ap, cur[:, j, :],
                        start=(first and j == 0), stop=(last and j == n - 1),
                    )

            def sel_for(b):
                return sel_a_t[:, BC - 1 - b : 2 * BC - 1 - b]

            # first article batch's DMA leads the queue
            t0 = load_batch(0)
            nc.sync.dma_start(sel_a_t[:], sel_a.ap()[:])
            nc.sync.dma_start(sel_o_t[:], sel_o.ap()[:])
            opt_t = dpool.tile([P, AWP, DIM], f32, tag="data")
            nc.sync.dma_start(opt_t[:], opt_r)

            # PE warmup: flip the HAM clock gate to 2.4 GHz early.
            for _ in range(WARMUP_MMS):
                nc.tensor.matmul(
                    psum_w[:], sel_o_t[:], sel_a_t[:], start=True, stop=True
                )

            reduce_tile(t0, AWP, sel_for(0), psum_a, True, False)

            # options; drain its psum into the output tile early
            reduce_tile(opt_t, AWP, sel_o_t[:], psum_b, True, True)
            nc.vector.tensor_copy(out_t[:, DIM : 2 * DIM], psum_b[:])

            for b in range(1, BC - 1):
                t = load_batch(b)
                reduce_tile(t, AWP, sel_for(b), psum_a, False, False)

            # final batch in geometrically shrinking chunks -> the very
            # last DMA is tiny and its fold+matmul tail is short
            sel_last = sel_for(BC - 1)
            assert sum(TAIL_CHUNKS) == AWP
            w0 = 0
            for i, nch in enumerate(TAIL_CHUNKS):
                t = dpool.tile([P, nch, DIM], f32, tag="data")
                nc.sync.dma_start(t[:], art_last[:, w0 : w0 + nch, :])
                reduce_tile(
                    t, nch, sel_last, psum_a, False, i == len(TAIL_CHUNKS) - 1
                )
                w0 += nch

            nc.vector.tensor_copy(out_t[:, 0:DIM], psum_a[:])
            nc.sync.dma_start(out.ap()[:], out_t[:])

    nc.compile()
    return nc


def get_nc():
    if "nc" not in _CACHE:
        _CACHE["nc"] = _build_nc()
    return _CACHE["nc"]


def _sel_arrays():
    # selector values carry the mean scaling (exact powers of two)
    sel_a = np.zeros((P, 2 * BC - 1), np.float32)
    sel_a[:, BC - 1] = 1.0 / AW
    sel_o = np.zeros((P, BC), np.float32)
    sel_o[np.arange(P), np.arange(P) // (P // BC)] = 1.0 / OW
    return sel_a, sel_o


def make_in_maps(article, options):
    article = np.ascontiguousarray(np.asarray(article, dtype=np.float32))
    options = np.ascontiguousarray(np.asarray(options, dtype=np.float32))
    assert article.shape == (B, AW, DIM), article.shape
    assert options.shape == (B, OW, DIM), options.shape
    sel_a, sel_o = _sel_arrays()
    return [
        {
            "article": article[i * BC : (i + 1) * BC],
            "options": options[i * BC : (i + 1) * BC],
            "sel_a": sel_a,
            "sel_o": sel_o,
        }
        for i in range(N_CORES)
    ]


def run_sharded(article, options, **spmd_kwargs):
    from concourse.bass_utils import run_bass_kernel_spmd

    nc = get_nc()
    in_maps = make_in_maps(article, options)
    res = run_bass_kernel_spmd(nc, in_maps, list(range(N_CORES)), **spmd_kwargs)
    full = np.concatenate(
        [res.results[i]["out"] for i in range(N_CORES)], axis=0
    ).astype(np.float32)
    return full, res


def kernel(article_concat, options_concat):
    full, _ = run_sharded(article_concat, options_concat)
    return full


# revision 12
# speedup vs baseline: 1.2815x; 1.2815x over previous
"""Trainium2 Bass kernel for nn_Concat_26147760898611.

Mean-pool over the word dim of article_concat [256, 2048, 300] and
options_concat [256, 64, 300], concat features -> [256, 600].

Sharding: pure data parallel over batch across 8 NeuronCores
(32 batches per core).

Per-core design (v5 — batch-pair tiles, fat 38.4 KB descriptors):
  - HWDGE descriptor dealing (measured): a DMA with n descriptors is
    dealt to k = (largest divisor of n <= 16) engines in consecutive
    chunks; only uniform 128-descriptor DMAs keep all 16 SDMA engines
    in their steady rhythm — any partial-engine DMA triggers a ~15 us
    global throughput dip (measured on v3/v4 of this kernel), so every
    data DMA here is exactly 128 descriptors.
  - Article batches are loaded in PAIRS: tile [128, 32, 300] where
    partition p holds words [32p, 32p+32) of the concatenated 2-batch
    stream — 38.4 KB contiguous per partition (double the descriptor
    size of the 1-batch layout, halving per-descriptor overhead, which
    is what limits the slowest SDMA engine).  2048 = 64*32, so
    partitions 0-63 hold batch 2t and 64-127 hold batch 2t+1 exactly.
  - Reduction per pair: DVE folds 32 -> 16 -> 8 -> 4, then 4 PE matmuls
    with a sliding TWO-hot selector (rows 0-63 -> psum row 2t, rows
    64-127 -> row 2t+1).  Selector values are 1/2048 (1/64 for options)
    so PSUM holds the mean directly and the Scalar engine (and its ACT
    table preamble load) is never used; DVE copies PSUM -> out tile.
  - Options: partition p holds 16 consecutive words of batch p//4, one
    block-selector reduction, drained into the output tile early.
  - The last pair is split into shrinking column chunks so the
    post-last-DMA tail (fold + matmul + copy + store) is short.
  - A burst of dummy matmuls at kernel start warms the PE HAM clock
    gate (1.2 -> 2.4 GHz) before real data lands.

Self-contained: hardcodes all shapes; no file reads.
"""

import numpy as np

N_CORES = 8
B = 256  # full batch
BC = B // N_CORES  # 32 batches per core
DIM = 300
AW = 2048  # article words per batch
OW = 64  # options words per batch
P = 128  # SBUF partitions
PAIRS = BC // 2  # 16 article batch-pairs per core
PW = 2 * AW // P  # 32 words per partition per pair

TAIL_CHUNKS = [16, 8, 4, 2, 1, 1]  # column split of the final pair
DATA_BUFS = 3
FOLD_BUFS = 2
WARMUP_MMS = 12

_CACHE = {}


def _build_nc():
    import concourse.bacc as bacc
    import concourse.mybir as mybir
    import concourse.tile as tile

    f32 = mybir.dt.float32
    nc = bacc.Bacc("TRN2", target_bir_lowering=False, debug=False)

    art = nc.dram_tensor("article", [BC, AW, DIM], f32, kind="ExternalInput")
    opt = nc.dram_tensor("options", [BC, OW, DIM], f32, kind="ExternalInput")
    sel_a = nc.dram_tensor("sel_a", [P, 2 * BC], f32, kind="ExternalInput")
    sel_o = nc.dram_tensor("sel_o", [P, BC], f32, kind="ExternalInput")
    out = nc.dram_tensor("out", [BC, 2 * DIM], f32, kind="ExternalOutput")

    # pair t view [128, 32, 300]: partition p <- words 32p..32p+31 of the
    # 4096-word pair stream (partitions 0-63 = batch 2t, 64-127 = 2t+1)
    art_flat = art.ap().rearrange("b w f -> (b w) f")

    def pair_view(t):
        return art_flat[t * 2 * AW : (t + 1) * 2 * AW].rearrange(
            "(p w) f -> p w f", p=P
        )

    # [128, 16, 300]: partition p <- 16 consecutive words of batch p//4
    opt_r = opt.ap().rearrange("b (s q) f -> (b s) q f", s=P // BC)

    with tile.TileContext(nc) as tc:
        with (
            tc.tile_pool(name="const", bufs=1) as cpool,
            tc.tile_pool(name="data", bufs=DATA_BUFS) as dpool,
            tc.tile_pool(name="fold", bufs=FOLD_BUFS) as fpool,
            tc.tile_pool(name="outp", bufs=1) as opool,
            tc.tile_pool(name="psum", bufs=1, space="PSUM") as ppool,
        ):
            sel_a_t = cpool.tile([P, 2 * BC], f32, tag="sel_a")
            sel_o_t = cpool.tile([P, BC], f32, tag="sel_o")
            out_t = opool.tile([BC, 2 * DIM], f32, tag="out")

            psum_a = ppool.tile([BC, DIM], f32, tag="psum_a")
            psum_b = ppool.tile([BC, DIM], f32, tag="psum_b")
            psum_w = ppool.tile([BC, BC], f32, tag="psum_w")

            def sel_pair(t):
                # window where col BC-2 -> psum row 2t, col BC-1 -> 2t+1
                return sel_a_t[:, BC - 2 - 2 * t : 2 * BC - 2 - 2 * t]

            def reduce_tile(t, nch, sel_ap, psum, first, last, fold_k=3):
                """Fold nch cols fold_k times on DVE, matmul-reduce the rest."""
                cur, n = t, nch
                for lvl in range(fold_k):
                    if n == 1:
                        break
                    n //= 2
                    nxt = fpool.tile([P, n, DIM], f32, tag=f"fold_{n}")
                    nc.vector.tensor_add(nxt[:], cur[:, 0:n, :], cur[:, n : 2 * n, :])
                    cur = nxt
                for j in range(n):
                    nc.tensor.matmul(
                        psum[:], sel_ap, cur[:, j, :],
                        start=(first and j == 0), stop=(last and j == n - 1),
                    )

            # first pair's DMA leads the queue
            t0 = dpool.tile([P, PW, DIM], f32, tag="data")
            nc.sync.dma_start(t0[:], pair_view(0))
            nc.sync.dma_start(sel_a_t[:], sel_a.ap()[:])
            nc.sync.dma_start(sel_o_t[:], sel_o.ap()[:])
            opt_t = dpool.tile([P, PW, DIM], f32, tag="data")
            nc.sync.dma_start(opt_t[:, 0:16, :], opt_r)

            # PE warmup: flip the HAM clock gate to 2.4 GHz early.
            for _ in range(WARMUP_MMS):
                nc.tensor.matmul(
                    psum_w[:], sel_o_t[:], sel_a_t[:, 0:BC], start=True, stop=True
                )

            reduce_tile(t0, PW, sel_pair(0), psum_a, True, False)

            # options; drain its psum into the output tile early
            reduce_tile(opt_t[:, 0:16, :], 16, sel_o_t[:], psum_b, True, True,
                        fold_k=2)
            nc.vector.tensor_copy(out_t[:, DIM : 2 * DIM], psum_b[:])
            nc.sync.dma_start(out.ap()[:, DIM : 2 * DIM], out_t[:, DIM : 2 * DIM])

            for t in range(1, PAIRS - 1):
                tl = dpool.tile([P, PW, DIM], f32, tag="data")
                nc.sync.dma_start(tl[:], pair_view(t))
                reduce_tile(tl, PW, sel_pair(t), psum_a, False, False)

            # final pair in shrinking column chunks -> the very last DMA is
            # small and its fold+matmul tail is short
            tp = PAIRS - 1
            pv = pair_view(tp)
            sel_last = sel_pair(tp)
            assert sum(TAIL_CHUNKS) == PW
            c0 = 0
            for i, nch in enumerate(TAIL_CHUNKS):
                tl = dpool.tile([P, nch, DIM], f32, tag="data")
                nc.sync.dma_start(tl[:], pv[:, c0 : c0 + nch, :])
                reduce_tile(
                    tl, nch, sel_last, psum_a, False, i == len(TAIL_CHUNKS) - 1,
                    fold_k=2,
                )
                c0 += nch

            nc.vector.tensor_copy(out_t[:, 0:DIM], psum_a[:])
            nc.sync.dma_start(out.ap()[:, 0:DIM], out_t[:, 0:DIM])

    nc.compile()
    return nc


def get_nc():
    if "nc" not in _CACHE:
        _CACHE["nc"] = _build_nc()
    return _CACHE["nc"]


def _sel_arrays():
    # selector values carry the mean scaling (exact powers of two).
    # sel_a: two-hot sliding selector for batch pairs — window
    # [BC-2-2t, 2*BC-2-2t) puts col BC-2 at psum row 2t (partitions 0-63)
    # and col BC-1 at row 2t+1 (partitions 64-127).
    sel_a = np.zeros((P, 2 * BC), np.float32)
    sel_a[0:64, BC - 2] = 1.0 / AW
    sel_a[64:P, BC - 1] = 1.0 / AW
    sel_o = np.zeros((P, BC), np.float32)
    sel_o[np.arange(P), np.arange(P) // (P // BC)] = 1.0 / OW
    return sel_a, sel_o


def make_in_maps(article, options):
    article = np.ascontiguousarray(np.asarray(article, dtype=np.float32))
    options = np.ascontiguousarray(np.asarray(options, dtype=np.float32))
    assert article.shape == (B, AW, DIM), article.shape
    assert options.shape == (B, OW, DIM), options.shape
    sel_a, sel_o = _sel_arrays()
    return [
        {
            "article": article[i * BC : (i + 1) * BC],
            "options": options[i * BC : (i + 1) * BC],
            "sel_a": sel_a,
            "sel_o": sel_o,
        }
        for i in range(N_CORES)
    ]


def run_sharded(article, options, **spmd_kwargs):
    from concourse.bass_utils import run_bass_kernel_spmd

    nc = get_nc()
    in_maps = make_in_maps(article, options)
    res = run_bass_kernel_spmd(nc, in_maps, list(range(N_CORES)), **spmd_kwargs)
    full = np.concatenate(
        [res.results[i]["out"] for i in range(N_CORES)], axis=0
    ).astype(np.float32)
    return full, res


def kernel(article_concat, options_concat):
    full, _ = run_sharded(article_concat, options_concat)
    return full
